# revision 15
# baseline (speedup 1.0000x reference)
"""Trainium2 Bass kernel for STSBaselineNet (embed -> biLSTM -> max-pool).

Sharding: one LSTM direction per core. Cores 0-3 run the forward pass of
sentence blocks 0-3; cores 4-7 run the backward pass of the same blocks
(time reversal and pad masking folded into host prep: reversed token order
plus a -BIG pad-flag lane on the i/f/o logits for the bwd cores).

Per core (64 sentences, one direction):
  Phase A: indirect-DMA gather of embedding rows in TIME-MAJOR token order
           (bf16, 384-feature rows: col 300 = 1.0 bias lane, col 301 = pad
           flag), PE transpose to feature-major, input projection into
           SBUF-resident zx. Time-major order makes every PSUM->zx copy a
           [128, 8x64-run] near-contiguous copy instead of a scatter.
  Phase B: 64-step recurrence. Gates on partitions (slices ordered
           i,i,f,f,o,o,g,g), sentences on the free dim (64 wide). zx is
           injected into the gate PSUM by an identity matmul so the DVE
           never touches the zx add. Elementwise uses merged full-width
           instructions: sigmoid[384], tanh[128], fused [i|f]*[g|c] mul,
           c-add, tanh(c), o*tanh(c) -> h (contiguous step-major store).
           A few dummy matmuls after each real block keep the PE activity
           monitor from clock-gating the array to half rate.
  Phase C: bulk mask add + max over time, PE transpose, DMA out [64, 256].
"""

import numpy as np
import ml_dtypes

import concourse.bass as bass
import concourse.bacc as bacc
import concourse.mybir as mybir
import concourse.tile as tile
from concourse import bass_utils

V, E, HID, B, T = 50000, 300, 256, 256, 64
NCORES = 8
NSC = 64                    # sentences per core (one direction)
NTOK = NSC * T              # 4096 tokens/core
NTT = NTOK // 128           # 32 gather tiles
EP = 384                    # padded feature dim (300 emb + bias + flag + 0pad)
BIGNEG = -30.0              # logit offset for gate masking (bwd cores)
MAXNEG = -8.0               # mask offset for the final max (|h| < 1)
NDUMMY = 3                  # warm-up matmuls per recurrence step

F32 = mybir.dt.float32
BF16 = mybir.dt.bfloat16
I32 = mybir.dt.int32
AF = mybir.ActivationFunctionType
OP = mybir.AluOpType

bf = ml_dtypes.bfloat16

# gate blocks [i, f, o, g]; torch row order in W is [i, f, g, o] (256 each).
GB_BASE = {0: 0, 1: 256, 2: 768, 3: 512}

_CACHE = {}
LAST_RESULTS = None


def _build_program():
    nc = bacc.Bacc(None, target_bir_lowering=False)

    emb_d = nc.dram_tensor("emb", [V, EP], BF16, kind="ExternalInput")
    idx_d = nc.dram_tensor("idx", [128, NTT], I32, kind="ExternalInput")
    mflag_d = nc.dram_tensor("mflag", [128, NTT], BF16, kind="ExternalInput")
    wstat_d = nc.dram_tensor("wstat", [128, 2048], BF16, kind="ExternalInput")
    wih_d = nc.dram_tensor("wih", [128, 3072], BF16, kind="ExternalInput")
    mbig_d = nc.dram_tensor("mbig", [128, 8192], BF16, kind="ExternalInput")
    out_d = nc.dram_tensor("out", [NSC, HID], F32, kind="ExternalOutput")

    with tile.TileContext(nc) as tc:
        with (
            tc.tile_pool(name="const", bufs=1) as cpool,
            tc.tile_pool(name="work", bufs=2) as wpool,
            tc.tile_pool(name="psump", bufs=2, space="PSUM") as ppool,
            tc.tile_pool(name="psumt", bufs=2, space="PSUM") as tpool,
            tc.tile_pool(name="psumif", bufs=2, space="PSUM") as ifpool,
            tc.tile_pool(name="psumg", bufs=1, space="PSUM") as gpool,
            tc.tile_pool(name="psumo", bufs=1, space="PSUM") as opool,
        ):
            dpool = ppool  # phase A's projection banks, reused for dummies
            wstat_sb = cpool.tile([128, 2048], BF16, tag="wstat")
            wih_sb = cpool.tile([128, 3072], BF16, tag="wih")
            idx_sb = cpool.tile([128, NTT], I32, tag="idx")
            mflag_sb = cpool.tile([128, NTT, 1], BF16, tag="mflag")
            mbig_sb = cpool.tile([128, 8192], BF16, tag="mbig")
            xg = cpool.tile([128, NTT * EP], BF16, tag="xg")
            xt = cpool.tile([128, 3 * NTOK], BF16, tag="xt")
            zx = cpool.tile([128, T * 512], BF16, tag="zx")
            # h(s) at cols (s+1)*128 + k*64 + b; cols 0:128 = h(-1) = 0
            h_all = cpool.tile([128, (T + 1) * 128], BF16, tag="h_all")
            # 0:384 sig(i,f,o) | 384:512 tanh(g) | 512:640 c (persistent)
            sgc = cpool.tile([128, 640], F32, tag="sgc")
            t0 = cpool.tile([128, 256], F32, tag="t0")
            tcv = cpool.tile([128, 128], F32, tag="tcv")
            ident = cpool.tile([128, 128], F32, tag="ident")
            ident_bf = cpool.tile([128, 128], BF16, tag="ident_bf")
            hmax = cpool.tile([128, 128], F32, tag="hmax")
            hmaxT = cpool.tile([128, 128], F32, tag="hmaxT")

            nc.sync.dma_start(out=idx_sb[:], in_=idx_d[:, :])
            nc.sync.dma_start(
                out=mflag_sb[:],
                in_=mflag_d[:, :].rearrange("p (t o) -> p t o", o=1))
            nc.sync.dma_start(out=wih_sb[:], in_=wih_d[:, :])
            nc.sync.dma_start(out=wstat_sb[:], in_=wstat_d[:, :])
            nc.sync.dma_start(out=mbig_sb[:], in_=mbig_d[:, :])

            nc.vector.memset(h_all[:, 0:128], 0.0)
            nc.vector.memset(sgc[:, 512:640], 0.0)
            nc.vector.memset(hmax[:], -30.0)
            from concourse.masks import make_identity
            make_identity(nc, ident[:])
            nc.vector.tensor_copy(out=ident_bf[:], in_=ident[:])

            # ---- Phase A building blocks ----
            # token j = s*64 + b (time-major); tile tk holds j in
            # [tk*128, (tk+1)*128), partition p = j - tk*128.
            # All of phase A is interleaved into the recurrence steps below
            # so the tensor/vector queues never sit ahead of the scan.
            xgv = xg[:].rearrange("p (tk f) -> p tk f", tk=NTT)
            zx_v = zx[:].rearrange("p (s c b) -> p s c b", s=T, c=8)
            ngrp = NTT // 4   # 8 groups of 4 tiles = 512 tokens each

            def gather_grp(grp):
                # gathers + lane augment all on the GpSimd queue: they never
                # block the vector/scalar queues that run the recurrence.
                for q in range(4):
                    tk = grp * 4 + q
                    nc.gpsimd.indirect_dma_start(
                        out=xg[:, tk * EP:(tk + 1) * EP],
                        out_offset=None,
                        in_=emb_d[:, :],
                        in_offset=bass.IndirectOffsetOnAxis(
                            ap=idx_sb[:, tk:tk + 1], axis=0),
                    )
                g4 = grp * 4
                nc.gpsimd.memset(xgv[:, g4:g4 + 4, 300:301], 1.0)
                nc.gpsimd.tensor_copy(
                    out=xgv[:, g4:g4 + 4, 301:302],
                    in_=mflag_sb[:, g4:g4 + 4, :])

            def transpose_mms(grp, kb):
                xtp = tpool.tile([128, 512], BF16, tag="xtp")
                for q in range(4):
                    tk = grp * 4 + q
                    nc.tensor.transpose(
                        xtp[:, q * 128:(q + 1) * 128],
                        xg[:, tk * EP + kb * 128:tk * EP + (kb + 1) * 128],
                        ident_bf[:])
                return xtp

            def transpose_copy(xtp, grp, kb):
                dst = xt[:, kb * NTOK + grp * 512:kb * NTOK + (grp + 1) * 512]
                if kb % 2 == 0:
                    nc.vector.tensor_copy(out=dst, in_=xtp[:])
                else:
                    nc.scalar.copy(out=dst, in_=xtp[:])

            def transpose_chunk(grp, kb):
                transpose_copy(transpose_mms(grp, kb), grp, kb)

            def proj_mms(n, ch):
                # psum col = s_loc*64 + b for token group n; zx col =
                # s*512 + ch*64 + b
                zxp = ppool.tile([128, 512], F32, tag="zxp")
                for kb in range(3):
                    nc.tensor.matmul(
                        zxp[:],
                        lhsT=wih_sb[:, (ch * 3 + kb) * 128:
                                    (ch * 3 + kb + 1) * 128],
                        rhs=xt[:, kb * NTOK + n * 512:kb * NTOK + (n + 1) * 512],
                        start=(kb == 0), stop=(kb == 2),
                    )
                return zxp

            def proj_copy(zxp, n, ch, on_vector):
                src = zxp[:].rearrange("p (s b) -> p s b", s=8)
                dst = zx_v[:, n * 8:(n + 1) * 8, ch, :]
                if on_vector:
                    nc.vector.tensor_copy(out=dst, in_=src)
                else:
                    nc.scalar.copy(out=dst, in_=src)

            def proj_chunk(n, ch, on_vector):
                proj_copy(proj_mms(n, ch), n, ch, on_vector)

            # prologue: everything needed for steps 0-7 plus a head start
            # on the gathers (their queue runs far ahead on its own).
            for grp in range(ngrp):
                gather_grp(grp)
            for grp in range(2):
                for kb in range(3):
                    transpose_chunk(grp, kb)
            for ch in range(8):
                proj_chunk(0, ch, ch % 2 == 0)

            # ---- Phase B: recurrence ----
            # gate slices: 0-3 = i,f (bank_if), 4-5 = o (bank_o),
            # 6-7 = g (bank_g). Each bank's accumulation group closes as
            # soon as its own matmuls finish, so activations start early.
            def gate_mms(bank, sl0, nsl, zx_lo, zx_hi, s):
                nc.tensor.matmul(
                    bank[:], lhsT=ident_bf[:],
                    rhs=zx[:, s * 512 + zx_lo:s * 512 + zx_hi],
                    start=True, stop=False,
                )
                for i in range(nsl):
                    sl = sl0 + i
                    for k in range(2):
                        nc.tensor.matmul(
                            bank[:, i * 64:(i + 1) * 64],
                            lhsT=wstat_sb[:, (sl * 2 + k) * 128:
                                          (sl * 2 + k + 1) * 128],
                            rhs=h_all[:, s * 128 + k * 64:s * 128 + (k + 1) * 64],
                            start=False, stop=(k == 1),
                        )

            for s in range(T):
                bg = gpool.tile([128, 128], F32, tag="zg")
                bif = ifpool.tile([128, 256], F32, tag="zif")
                bo = opool.tile([128, 128], F32, tag="zo")
                gate_mms(bg, 6, 2, 384, 512, s)    # g first: unblocks TANHg
                gate_mms(bif, 0, 4, 0, 256, s)
                gate_mms(bo, 4, 2, 256, 384, s)
                # phase-A work for later steps fills this step's elementwise
                # window on the tensor queue (emitted after the gate MMs, so
                # it runs while the chain is on the vector/scalar engines).
                # The PSUM->SBUF copies are emitted after the chain ops so
                # they queue behind them on the vector/scalar engines.
                pj = tr = None
                if s < 56:
                    pj = proj_mms(s // 8 + 1, s % 8)
                if s % 8 < 3 and s // 8 + 2 < ngrp:
                    tr = transpose_mms(s // 8 + 2, s % 8)
                # dummy matmuls keep the PE activity monitor from gating
                # the clock once no phase-A work is left to fill the window.
                if NDUMMY and s >= 56:
                    dmy = dpool.tile([128, 512], F32, tag="zxp")
                    for dd in range(NDUMMY):
                        nc.tensor.matmul(
                            dmy[:], lhsT=ident_bf[:],
                            rhs=zx[:, s * 512:(s + 1) * 512],
                            start=(dd == 0), stop=(dd == NDUMMY - 1),
                        )
                # elementwise: sgc = [sig(i,f) | sig(o) | tanh(g) | c]
                nc.scalar.activation(sgc[:, 384:512], bg[:], AF.Tanh)
                nc.scalar.activation(sgc[:, 0:256], bif[:], AF.Sigmoid)
                nc.vector.tensor_mul(t0[:], sgc[:, 0:256], sgc[:, 384:640])
                nc.scalar.activation(sgc[:, 256:384], bo[:], AF.Sigmoid)
                nc.vector.tensor_add(sgc[:, 512:640], t0[:, 0:128], t0[:, 128:256])
                nc.scalar.activation(tcv[:], sgc[:, 512:640], AF.Tanh)
                nc.vector.tensor_mul(
                    h_all[:, (s + 1) * 128:(s + 2) * 128],
                    sgc[:, 256:384], tcv[:])
                # running masked max (off the critical chain)
                hm = wpool.tile([128, 128], F32, tag="hm")
                nc.vector.tensor_add(
                    hm[:], h_all[:, (s + 1) * 128:(s + 2) * 128],
                    mbig_sb[:, s * 128:(s + 1) * 128])
                nc.vector.tensor_max(hmax[:], hmax[:], hm[:])
                # phase-A copies go last in the engine queues
                if pj is not None:
                    proj_copy(pj, s // 8 + 1, s % 8, s % 2 == 0)
                if tr is not None:
                    transpose_copy(tr, s // 8 + 2, s % 8)

            # ---- Phase C: output ----
            tp = opool.tile([128, 128], F32, tag="zo")
            nc.tensor.transpose(tp[:], hmax[:], ident[:])
            nc.vector.tensor_copy(out=hmaxT[:], in_=tp[:])
            # out[b, k*128 + p] <- hmaxT[j = k*64 + b, p]
            out_ap = bass.AP(tensor=out_d[:, :].tensor, offset=0,
                             ap=[[128, 2], [HID, NSC], [1, 128]])
            nc.sync.dma_start(out=out_ap, in_=hmaxT[:])

    nc.finalize()
    return nc


def _sel_rows(ch):
    gb, ko = ch // 2, ch % 2
    base = GB_BASE[gb] + ko * 128
    return slice(base, base + 128)


def _host_prep(token_ids, lengths, emb, w_ih_f, w_hh_f, b_f, w_ih_b, w_hh_b,
               b_b):
    emb384 = np.zeros((V, EP), dtype=bf)
    emb384[:, :E] = emb.astype(bf)

    wstat_d, wih_d = {}, {}
    for d in range(2):
        whh = w_hh_f if d == 0 else w_hh_b
        wstat = np.zeros((128, 2048), dtype=bf)
        for sl in range(8):
            for k in range(2):
                blk = whh[_sel_rows(sl), k * 128:(k + 1) * 128].T
                col = (sl * 2 + k) * 128
                wstat[:, col:col + 128] = blk.astype(bf)
        wstat_d[d] = wstat

        w_ih = w_ih_f if d == 0 else w_ih_b
        bias = b_f if d == 0 else b_b
        aug = np.zeros((EP, 4 * HID), dtype=np.float32)
        aug[:E, :] = w_ih.T
        aug[300, :] = bias
        if d == 1:
            mv = np.zeros(4 * HID, dtype=np.float32)
            mv[0:512] = BIGNEG          # i, f
            mv[768:1024] = BIGNEG       # o
            aug[301, :] = mv
        wih = np.zeros((128, 3072), dtype=bf)
        for ch in range(8):
            for kb in range(3):
                blk = aug[kb * 128:(kb + 1) * 128, _sel_rows(ch)]
                col = (ch * 3 + kb) * 128
                wih[:, col:col + 128] = blk.astype(bf)
        wih_d[d] = wih

    in_maps = []
    for c in range(NCORES):
        d = 0 if c < 4 else 1
        blk = c % 4
        tok = token_ids[blk * NSC:(blk + 1) * NSC]      # [64, 64]
        ln = lengths[blk * NSC:(blk + 1) * NSC]         # [64]
        if d == 1:
            tok = tok[:, ::-1]                          # scan order = reversed

        flat = tok.T.reshape(-1)                        # j = s*64 + b
        idx = flat.reshape(NTT, 128).T.astype(np.int32).copy()

        ss = np.arange(T)[None, :]
        t_of_s = ss if d == 0 else T - 1 - ss
        pad = (t_of_s >= ln[:, None]).astype(np.float32)   # [64 b, 64 s]
        mflag = pad.T.reshape(-1).reshape(NTT, 128).T.astype(bf).copy()

        # mbig[p, s*128 + k*64 + b] = MAXNEG on pad steps
        mb_row = np.where(pad.T[:, None, :], MAXNEG, 0.0)   # [s, 1, b]
        mb_row = np.broadcast_to(mb_row, (T, 2, NSC)).reshape(-1)
        mb_ = np.broadcast_to(mb_row[None, :], (128, 8192))
        in_maps.append({
            "emb": emb384,
            "idx": idx,
            "mflag": mflag,
            "wstat": wstat_d[d],
            "wih": wih_d[d],
            "mbig": mb_.astype(bf),
        })
    return in_maps


def kernel(token_ids, lengths, emb, w_ih_f, w_hh_f, b_f, w_ih_b, w_hh_b, b_b):
    global LAST_RESULTS
    if "nc" not in _CACHE:
        _CACHE["nc"] = _build_program()
    nc = _CACHE["nc"]
    in_maps = _host_prep(token_ids, lengths, emb, w_ih_f, w_hh_f, b_f,
                         w_ih_b, w_hh_b, b_b)
    res = bass_utils.run_bass_kernel_spmd(nc, in_maps, list(range(NCORES)))
    LAST_RESULTS = res
    out = np.zeros((B, 2 * HID), np.float32)
    for c in range(NCORES):
        d = 0 if c < 4 else 1
        blk = c % 4
        out[blk * NSC:(blk + 1) * NSC,
            d * HID:(d + 1) * HID] = res.results[c]["out"]
    return out


# revision 18
# speedup vs baseline: 1.2323x; 1.2323x over previous
"""Trainium2 Bass kernel for STSBaselineNet (embed -> biLSTM -> max-pool).

Sharding: one LSTM direction per core. Cores 0-3 run the forward pass of
sentence blocks 0-3; cores 4-7 run the backward pass of the same blocks
(time reversal and pad masking folded into host prep: reversed token order
plus a -BIG pad-flag lane on the i/f/o logits for the bwd cores).

Per core (64 sentences, one direction):
  Phase A: indirect-DMA gather of embedding rows in TIME-MAJOR token order
           (bf16, 384-feature rows: col 300 = 1.0 bias lane, col 301 = pad
           flag), PE transpose to feature-major, input projection into
           SBUF-resident zx. Time-major order makes every PSUM->zx copy a
           [128, 8x64-run] near-contiguous copy instead of a scatter.
  Phase B: 64-step recurrence. Gates on partitions (slices ordered
           i,i,f,f,o,o,g,g), sentences on the free dim (64 wide). zx is
           injected into the gate PSUM by an identity matmul so the DVE
           never touches the zx add. Elementwise uses merged full-width
           instructions: sigmoid[384], tanh[128], fused [i|f]*[g|c] mul,
           c-add, tanh(c), o*tanh(c) -> h (contiguous step-major store).
           A few dummy matmuls after each real block keep the PE activity
           monitor from clock-gating the array to half rate.
  Phase C: bulk mask add + max over time, PE transpose, DMA out [64, 256].
"""

import numpy as np
import ml_dtypes

import concourse.bass as bass
import concourse.bacc as bacc
import concourse.mybir as mybir
import concourse.tile as tile
from concourse import bass_utils

V, E, HID, B, T = 50000, 300, 256, 256, 64
NCORES = 8
NSC = 64                    # sentences per core (one direction)
NTOK = NSC * T              # 4096 tokens/core
NTT = NTOK // 128           # 32 gather tiles
EP = 384                    # padded feature dim (300 emb + bias + flag + 0pad)
BIGNEG = -30.0              # logit offset for gate masking (bwd cores)
MAXNEG = -8.0               # mask offset for the final max (|h| < 1)
NDUMMY = 3                  # warm-up matmuls per recurrence step

F32 = mybir.dt.float32
BF16 = mybir.dt.bfloat16
I32 = mybir.dt.int32
AF = mybir.ActivationFunctionType
OP = mybir.AluOpType

bf = ml_dtypes.bfloat16

# gate blocks [i, f, o, g]; torch row order in W is [i, f, g, o] (256 each).
GB_BASE = {0: 0, 1: 256, 2: 768, 3: 512}

_CACHE = {}
LAST_RESULTS = None


def _build_program():
    nc = bacc.Bacc(None, target_bir_lowering=False)

    emb_d = nc.dram_tensor("emb", [V, EP], BF16, kind="ExternalInput")
    idx_d = nc.dram_tensor("idx", [128, NTT], I32, kind="ExternalInput")
    mflag_d = nc.dram_tensor("mflag", [128, NTT], BF16, kind="ExternalInput")
    wstat_d = nc.dram_tensor("wstat", [128, 2048], BF16, kind="ExternalInput")
    wih_d = nc.dram_tensor("wih", [128, 3072], BF16, kind="ExternalInput")
    mbig_d = nc.dram_tensor("mbig", [128, 8192], BF16, kind="ExternalInput")
    out_d = nc.dram_tensor("out", [NSC, HID], F32, kind="ExternalOutput")

    with tile.TileContext(nc) as tc:
        with (
            tc.tile_pool(name="const", bufs=1) as cpool,
            tc.tile_pool(name="work", bufs=2) as wpool,
            tc.tile_pool(name="psump", bufs=2, space="PSUM") as ppool,
            tc.tile_pool(name="psumt", bufs=2, space="PSUM") as tpool,
            tc.tile_pool(name="psumif", bufs=2, space="PSUM") as ifpool,
            tc.tile_pool(name="psumg", bufs=1, space="PSUM") as gpool,
            tc.tile_pool(name="psumo", bufs=1, space="PSUM") as opool,
        ):
            dpool = ppool  # phase A's projection banks, reused for dummies
            wstat_sb = cpool.tile([128, 2048], BF16, tag="wstat")
            wih_sb = cpool.tile([128, 3072], BF16, tag="wih")
            idx_sb = cpool.tile([128, NTT], I32, tag="idx")
            mflag_sb = cpool.tile([128, NTT, 1], BF16, tag="mflag")
            mbig_sb = cpool.tile([128, 8192], BF16, tag="mbig")
            xg = cpool.tile([128, NTT * EP], BF16, tag="xg")
            xt = cpool.tile([128, 3 * NTOK], BF16, tag="xt")
            zx = cpool.tile([128, T * 512], BF16, tag="zx")
            # h(s) at cols (s+1)*128 + k*64 + b; cols 0:128 = h(-1) = 0
            h_all = cpool.tile([128, (T + 1) * 128], BF16, tag="h_all")
            # 0:384 sig(i,f,o) | 384:512 tanh(g) | 512:640 c (persistent)
            sgc = cpool.tile([128, 640], F32, tag="sgc")
            t0 = cpool.tile([128, 256], F32, tag="t0")
            tcv = cpool.tile([128, 128], F32, tag="tcv")
            ident = cpool.tile([128, 128], F32, tag="ident")
            ident_bf = cpool.tile([128, 128], BF16, tag="ident_bf")
            hmax = cpool.tile([128, 128], F32, tag="hmax")
            hmaxT = cpool.tile([128, 128], F32, tag="hmaxT")

            nc.sync.dma_start(out=idx_sb[:], in_=idx_d[:, :])
            nc.sync.dma_start(
                out=mflag_sb[:],
                in_=mflag_d[:, :].rearrange("p (t o) -> p t o", o=1))
            nc.sync.dma_start(out=wih_sb[:], in_=wih_d[:, :])
            nc.sync.dma_start(out=wstat_sb[:], in_=wstat_d[:, :])
            nc.sync.dma_start(out=mbig_sb[:], in_=mbig_d[:, :])

            nc.vector.memset(h_all[:, 0:128], 0.0)
            nc.vector.memset(sgc[:, 512:640], 0.0)
            nc.vector.memset(hmax[:], -30.0)
            from concourse.masks import make_identity
            make_identity(nc, ident[:])
            nc.vector.tensor_copy(out=ident_bf[:], in_=ident[:])
            # preload both ACT tables off the critical path (the tanh
            # table otherwise loads lazily right before step 0's TANH)
            nc.scalar.activation(tcv[:, 0:1], ident[:, 0:1], AF.Tanh)
            nc.scalar.activation(tcv[:, 1:2], ident[:, 0:1], AF.Sigmoid)
            # pre-warm the PE clock (HAM) while the first gathers run
            warm = ppool.tile([128, 512], F32, tag="zxp")
            for w in range(24):
                nc.tensor.matmul(warm[:, 0:128], lhsT=ident_bf[:],
                                 rhs=ident_bf[:], start=True, stop=True)

            # ---- Phase A building blocks ----
            # token j = s*64 + b (time-major); tile tk holds j in
            # [tk*128, (tk+1)*128), partition p = j - tk*128.
            # All of phase A is interleaved into the recurrence steps below
            # so the tensor/vector queues never sit ahead of the scan.
            xgv = xg[:].rearrange("p (tk f) -> p tk f", tk=NTT)
            zx_v = zx[:].rearrange("p (s c b) -> p s c b", s=T, c=8)
            ngrp = NTT // 4   # 8 groups of 4 tiles = 512 tokens each

            def gather_grp(grp):
                # gathers + lane augment all on the GpSimd queue: they never
                # block the vector/scalar queues that run the recurrence.
                # Group 0 augments per tile so tile 0 unblocks immediately.
                for q in range(4):
                    tk = grp * 4 + q
                    nc.gpsimd.indirect_dma_start(
                        out=xg[:, tk * EP:(tk + 1) * EP],
                        out_offset=None,
                        in_=emb_d[:, :],
                        in_offset=bass.IndirectOffsetOnAxis(
                            ap=idx_sb[:, tk:tk + 1], axis=0),
                    )
                    if grp == 0:
                        nc.gpsimd.memset(xgv[:, tk:tk + 1, 300:301], 1.0)
                        nc.gpsimd.tensor_copy(
                            out=xgv[:, tk:tk + 1, 301:302],
                            in_=mflag_sb[:, tk:tk + 1, :])
                if grp > 0:
                    g4 = grp * 4
                    nc.gpsimd.memset(xgv[:, g4:g4 + 4, 300:301], 1.0)
                    nc.gpsimd.tensor_copy(
                        out=xgv[:, g4:g4 + 4, 301:302],
                        in_=mflag_sb[:, g4:g4 + 4, :])

            def transpose_mms(grp, kb):
                xtp = tpool.tile([128, 512], BF16, tag="xtp")
                for q in range(4):
                    tk = grp * 4 + q
                    nc.tensor.transpose(
                        xtp[:, q * 128:(q + 1) * 128],
                        xg[:, tk * EP + kb * 128:tk * EP + (kb + 1) * 128],
                        ident_bf[:])
                return xtp

            def transpose_copy(xtp, grp, kb):
                dst = xt[:, kb * NTOK + grp * 512:kb * NTOK + (grp + 1) * 512]
                if kb % 2 == 0:
                    nc.vector.tensor_copy(out=dst, in_=xtp[:])
                else:
                    nc.scalar.copy(out=dst, in_=xtp[:])

            def transpose_chunk(grp, kb):
                transpose_copy(transpose_mms(grp, kb), grp, kb)

            def proj_mms(n, ch):
                # psum col = s_loc*64 + b for token group n; zx col =
                # s*512 + ch*64 + b
                zxp = ppool.tile([128, 512], F32, tag="zxp")
                for kb in range(3):
                    nc.tensor.matmul(
                        zxp[:],
                        lhsT=wih_sb[:, (ch * 3 + kb) * 128:
                                    (ch * 3 + kb + 1) * 128],
                        rhs=xt[:, kb * NTOK + n * 512:kb * NTOK + (n + 1) * 512],
                        start=(kb == 0), stop=(kb == 2),
                    )
                return zxp

            def proj_copy(zxp, n, ch, on_vector):
                src = zxp[:].rearrange("p (s b) -> p s b", s=8)
                dst = zx_v[:, n * 8:(n + 1) * 8, ch, :]
                if on_vector:
                    nc.vector.tensor_copy(out=dst, in_=src)
                else:
                    nc.scalar.copy(out=dst, in_=src)

            def proj_chunk(n, ch, on_vector):
                proj_copy(proj_mms(n, ch), n, ch, on_vector)

            # prologue: fine-grained head so step 0 starts as soon as the
            # FIRST gather tile (tokens of steps 0-1) lands, instead of
            # waiting for the whole first group.
            for grp in range(ngrp):
                gather_grp(grp)

            # tile 0: transpose + project steps 0-1 (N=128)
            xtp0 = tpool.tile([128, 512], BF16, tag="xtp")
            for kb in range(3):
                nc.tensor.transpose(
                    xtp0[:, kb * 128:(kb + 1) * 128],
                    xg[:, kb * 128:(kb + 1) * 128], ident_bf[:])
            for kb in range(3):
                nc.vector.tensor_copy(
                    out=xt[:, kb * NTOK:kb * NTOK + 128],
                    in_=xtp0[:, kb * 128:(kb + 1) * 128])
            for half in range(2):
                zxp0 = ppool.tile([128, 512], F32, tag="zxp")
                for q in range(4):
                    ch = half * 4 + q
                    for kb in range(3):
                        nc.tensor.matmul(
                            zxp0[:, q * 128:(q + 1) * 128],
                            lhsT=wih_sb[:, (ch * 3 + kb) * 128:
                                        (ch * 3 + kb + 1) * 128],
                            rhs=xt[:, kb * NTOK:kb * NTOK + 128],
                            start=(kb == 0), stop=(kb == 2),
                        )
                for q in range(4):
                    ch = half * 4 + q
                    src = zxp0[:, q * 128:(q + 1) * 128].rearrange(
                        "p (s b) -> p s b", s=2)
                    nc.vector.tensor_copy(out=zx_v[:, 0:2, ch, :], in_=src)

            # tiles 1-3: transpose, then project steps 2-7 (N=384)
            for tk in range(1, 4):
                xtpk = tpool.tile([128, 512], BF16, tag="xtp")
                for kb in range(3):
                    nc.tensor.transpose(
                        xtpk[:, kb * 128:(kb + 1) * 128],
                        xg[:, tk * EP + kb * 128:tk * EP + (kb + 1) * 128],
                        ident_bf[:])
                for kb in range(3):
                    dst = xt[:, kb * NTOK + tk * 128:kb * NTOK + (tk + 1) * 128]
                    if kb % 2 == 0:
                        nc.vector.tensor_copy(
                            out=dst, in_=xtpk[:, kb * 128:(kb + 1) * 128])
                    else:
                        nc.scalar.copy(
                            out=dst, in_=xtpk[:, kb * 128:(kb + 1) * 128])
            for ch in range(8):
                zxpk = ppool.tile([128, 512], F32, tag="zxp")
                for kb in range(3):
                    nc.tensor.matmul(
                        zxpk[:, 0:384],
                        lhsT=wih_sb[:, (ch * 3 + kb) * 128:
                                    (ch * 3 + kb + 1) * 128],
                        rhs=xt[:, kb * NTOK + 128:kb * NTOK + 512],
                        start=(kb == 0), stop=(kb == 2),
                    )
                src = zxpk[:, 0:384].rearrange("p (s b) -> p s b", s=6)
                if ch % 2 == 0:
                    nc.vector.tensor_copy(out=zx_v[:, 2:8, ch, :], in_=src)
                else:
                    nc.scalar.copy(out=zx_v[:, 2:8, ch, :], in_=src)

            # group 1 transposes (its projections interleave into steps 0-7)
            for kb in range(3):
                transpose_chunk(1, kb)

            # ---- Phase B: recurrence ----
            # gate slices: 0-3 = i,f (bank_if), 4-5 = o (bank_o),
            # 6-7 = g (bank_g). Each bank's accumulation group closes as
            # soon as its own matmuls finish, so activations start early.
            def gate_mms(bank, sl0, nsl, zx_lo, zx_hi, s):
                nc.tensor.matmul(
                    bank[:], lhsT=ident_bf[:],
                    rhs=zx[:, s * 512 + zx_lo:s * 512 + zx_hi],
                    start=True, stop=False,
                )
                for i in range(nsl):
                    sl = sl0 + i
                    for k in range(2):
                        nc.tensor.matmul(
                            bank[:, i * 64:(i + 1) * 64],
                            lhsT=wstat_sb[:, (sl * 2 + k) * 128:
                                          (sl * 2 + k + 1) * 128],
                            rhs=h_all[:, s * 128 + k * 64:s * 128 + (k + 1) * 64],
                            start=False, stop=(k == 1),
                        )

            for s in range(T):
                bg = gpool.tile([128, 128], F32, tag="zg")
                bif = ifpool.tile([128, 256], F32, tag="zif")
                bo = opool.tile([128, 128], F32, tag="zo")
                gate_mms(bg, 6, 2, 384, 512, s)    # g first: unblocks TANHg
                gate_mms(bif, 0, 4, 0, 256, s)
                gate_mms(bo, 4, 2, 256, 384, s)
                # phase-A work for later steps fills this step's elementwise
                # window on the tensor queue (emitted after the gate MMs, so
                # it runs while the chain is on the vector/scalar engines).
                # The PSUM->SBUF copies are emitted after the chain ops so
                # they queue behind them on the vector/scalar engines.
                pj = tr = None
                if s < 56:
                    pj = proj_mms(s // 8 + 1, s % 8)
                if s % 8 < 3 and s // 8 + 2 < ngrp:
                    tr = transpose_mms(s // 8 + 2, s % 8)
                # dummy matmuls keep the PE activity monitor from gating
                # the clock once no phase-A work is left to fill the window.
                if NDUMMY and s >= 56:
                    dmy = dpool.tile([128, 512], F32, tag="zxp")
                    for dd in range(NDUMMY):
                        nc.tensor.matmul(
                            dmy[:], lhsT=ident_bf[:],
                            rhs=zx[:, s * 512:(s + 1) * 512],
                            start=(dd == 0), stop=(dd == NDUMMY - 1),
                        )
                # elementwise: sgc = [sig(i,f) | sig(o) | tanh(g) | c]
                nc.scalar.activation(sgc[:, 384:512], bg[:], AF.Tanh)
                nc.scalar.activation(sgc[:, 0:256], bif[:], AF.Sigmoid)
                nc.vector.tensor_mul(t0[:], sgc[:, 0:256], sgc[:, 384:640])
                nc.scalar.activation(sgc[:, 256:384], bo[:], AF.Sigmoid)
                nc.vector.tensor_add(sgc[:, 512:640], t0[:, 0:128], t0[:, 128:256])
                nc.scalar.activation(tcv[:], sgc[:, 512:640], AF.Tanh)
                nc.vector.tensor_mul(
                    h_all[:, (s + 1) * 128:(s + 2) * 128],
                    sgc[:, 256:384], tcv[:])
                # running masked max (off the critical chain)
                hm = wpool.tile([128, 128], F32, tag="hm")
                nc.vector.tensor_add(
                    hm[:], h_all[:, (s + 1) * 128:(s + 2) * 128],
                    mbig_sb[:, s * 128:(s + 1) * 128])
                nc.vector.tensor_max(hmax[:], hmax[:], hm[:])
                # phase-A copies go last in the engine queues
                if pj is not None:
                    proj_copy(pj, s // 8 + 1, s % 8, s % 2 == 0)
                if tr is not None:
                    transpose_copy(tr, s // 8 + 2, s % 8)

            # ---- Phase C: output ----
            tp = opool.tile([128, 128], F32, tag="zo")
            nc.tensor.transpose(tp[:], hmax[:], ident[:])
            nc.vector.tensor_copy(out=hmaxT[:], in_=tp[:])
            # out[b, k*128 + p] <- hmaxT[j = k*64 + b, p]
            out_ap = bass.AP(tensor=out_d[:, :].tensor, offset=0,
                             ap=[[128, 2], [HID, NSC], [1, 128]])
            nc.sync.dma_start(out=out_ap, in_=hmaxT[:])

    nc.finalize()
    return nc


def _sel_rows(ch):
    gb, ko = ch // 2, ch % 2
    base = GB_BASE[gb] + ko * 128
    return slice(base, base + 128)


def _host_prep(token_ids, lengths, emb, w_ih_f, w_hh_f, b_f, w_ih_b, w_hh_b,
               b_b):
    emb384 = np.zeros((V, EP), dtype=bf)
    emb384[:, :E] = emb.astype(bf)

    wstat_d, wih_d = {}, {}
    for d in range(2):
        whh = w_hh_f if d == 0 else w_hh_b
        wstat = np.zeros((128, 2048), dtype=bf)
        for sl in range(8):
            for k in range(2):
                blk = whh[_sel_rows(sl), k * 128:(k + 1) * 128].T
                col = (sl * 2 + k) * 128
                wstat[:, col:col + 128] = blk.astype(bf)
        wstat_d[d] = wstat

        w_ih = w_ih_f if d == 0 else w_ih_b
        bias = b_f if d == 0 else b_b
        aug = np.zeros((EP, 4 * HID), dtype=np.float32)
        aug[:E, :] = w_ih.T
        aug[300, :] = bias
        if d == 1:
            mv = np.zeros(4 * HID, dtype=np.float32)
            mv[0:512] = BIGNEG          # i, f
            mv[768:1024] = BIGNEG       # o
            aug[301, :] = mv
        wih = np.zeros((128, 3072), dtype=bf)
        for ch in range(8):
            for kb in range(3):
                blk = aug[kb * 128:(kb + 1) * 128, _sel_rows(ch)]
                col = (ch * 3 + kb) * 128
                wih[:, col:col + 128] = blk.astype(bf)
        wih_d[d] = wih

    in_maps = []
    for c in range(NCORES):
        d = 0 if c < 4 else 1
        blk = c % 4
        tok = token_ids[blk * NSC:(blk + 1) * NSC]      # [64, 64]
        ln = lengths[blk * NSC:(blk + 1) * NSC]         # [64]
        if d == 1:
            tok = tok[:, ::-1]                          # scan order = reversed

        flat = tok.T.reshape(-1)                        # j = s*64 + b
        idx = flat.reshape(NTT, 128).T.astype(np.int32).copy()

        ss = np.arange(T)[None, :]
        t_of_s = ss if d == 0 else T - 1 - ss
        pad = (t_of_s >= ln[:, None]).astype(np.float32)   # [64 b, 64 s]
        mflag = pad.T.reshape(-1).reshape(NTT, 128).T.astype(bf).copy()

        # mbig[p, s*128 + k*64 + b] = MAXNEG on pad steps
        mb_row = np.where(pad.T[:, None, :], MAXNEG, 0.0)   # [s, 1, b]
        mb_row = np.broadcast_to(mb_row, (T, 2, NSC)).reshape(-1)
        mb_ = np.broadcast_to(mb_row[None, :], (128, 8192))
        in_maps.append({
            "emb": emb384,
            "idx": idx,
            "mflag": mflag,
            "wstat": wstat_d[d],
            "wih": wih_d[d],
            "mbig": mb_.astype(bf),
        })
    return in_maps


def kernel(token_ids, lengths, emb, w_ih_f, w_hh_f, b_f, w_ih_b, w_hh_b, b_b):
    global LAST_RESULTS
    if "nc" not in _CACHE:
        _CACHE["nc"] = _build_program()
    nc = _CACHE["nc"]
    in_maps = _host_prep(token_ids, lengths, emb, w_ih_f, w_hh_f, b_f,
                         w_ih_b, w_hh_b, b_b)
    res = bass_utils.run_bass_kernel_spmd(nc, in_maps, list(range(NCORES)))
    LAST_RESULTS = res
    out = np.zeros((B, 2 * HID), np.float32)
    for c in range(NCORES):
        d = 0 if c < 4 else 1
        blk = c % 4
        out[blk * NSC:(blk + 1) * NSC,
            d * HID:(d + 1) * HID] = res.results[c]["out"]
    return out
